# revision 24
# baseline (speedup 1.0000x reference)
"""GCN layer (segment-sum + linear) on 8 Trainium2 NeuronCores.

Reference computation:
    agg = segment_sum(x[src], dst, num_segments=N)   # (N, 128)
    out = agg @ W + b                                # (N, 128)

Strategy
--------
dst nodes are partitioned across the 8 cores (graph/data parallel), assigned
to (core, slot) by a degree-sorted snake deal so every 16-slot "window" of
every core receives a near-equal number of incoming edges. Each core
processes its 12.5K slots in blocks of 512 (one fp32 PSUM bank).

The host resolves the irregular per-edge gather at pack time: for each core
it materializes the edge-row sequence
    xs[c][p, t*128:(t+1)*128] = x_fp8e3m4[src(c, p, t)]
in exactly the (partition, tile) layout the segment-sum matmuls consume
(zero rows in pad slots), so the device streams edge rows CONTIGUOUSLY at
full HBM rate instead of issuing one small random-gather descriptor per
edge. Per 512-slot block the device runs:

  1. [128 x 512] block-wide straggler matmuls for overflow edges, issued
     FIRST — the leading one carries start=True, which zeroes the whole
     PSUM bank (every block is forced to have >= 1 straggler tile for
     exactly this reason; a second start=True inside an accumulation
     group wipes earlier columns, so there is exactly one per block),
  2. one TensorE matmul  agg[:, w*8:(w+1)*8] += Xs_tile.T @ S_tile per
     [128 x 8] window tile, where S is a host-built one-hot fp8e3m4
     matrix (S[e, j] = 1 iff edge e's dst sits at column j),
  3. the linear layer in fp16 (fp32 PSUM accum): out = agg.T @ W + b,
  4. one fp16 output DMA per 3-block group (host upcasts to fp32 after
     un-permuting).

All accumulation is fp32; the dominant rounding is fp8e3m4 (4 mantissa
bits) on x itself, ~1.3e-2 relative error against the fp32 reference —
inside the 2e-2 gate with margin, and it halves the xs HBM stream that
bounds this kernel. Queue layout keeps the xs stream unobstructed: xs
alone on the SP HWDGE queue, S on the gpsimd SWDGE queue, the output DMA
on the ACT HWDGE queue, PSUM->SBUF copies and bias adds on DVE.
"""

import math

import numpy as np

import concourse.bacc as bacc
import concourse.bass as bass
import concourse.mybir as mybir
from concourse import bass_utils
from concourse.tile import TileContext

NCORES = 8
PART = 128
BLK = 512
WIN = 8
D = 128
GRP = 3             # blocks per DMA group

F16 = mybir.dt.float16
F32 = mybir.dt.float32
F8 = mybir.dt.float8e3


def _sanitize_waits(nc):
    for bb in nc.m.functions[0].blocks:
        insts = list(bb.instructions)
        out = []
        changed = False
        for inst in insts:
            si = inst.sync_info
            waits = list(si.on_wait) if si is not None and si.on_wait else []
            if len(waits) > 1:
                changed = True
                eng = inst.engine
                for w in waits[:-1]:
                    nop = nc.engines[eng].nop(nofuse=True, hint="wsplit")
                    cb = nc.cur_bb.bb
                    cl = list(cb.instructions)
                    assert cl and cl[-1].name == nop.ins.name
                    cb.instructions = cl[:-1]
                    nop.ins.sync_info = mybir.SyncInfo(on_wait=[w], on_update=[])
                    out.append(nop.ins)
                inst.sync_info = mybir.SyncInfo(
                    on_wait=[waits[-1]], on_update=list(si.on_update or [])
                )
            out.append(inst)
        if changed:
            bb.instructions = out


def plan_schedule(npc, win=WIN):
    blocks = []
    for b0 in range(0, npc, BLK):
        ns = min(BLK, npc - b0)
        wins = []
        for w0 in range(0, ns, win):
            cap = min(win, ns - w0)
            nt = max(1, int(round(cap * 16.0 / 128.0)))
            wins.append((cap, nt))
        blocks.append((ns, wins))
    return blocks


def tbase_ord(wins, w):
    t = 0
    for i in range(w):
        t += wins[i][1]
    return t


def pack_inputs(x, edge_index, W, b, grp=GRP, win=WIN, out_f16=True):
    x = np.asarray(x)
    W = np.asarray(W, dtype=np.float32)
    bias = np.asarray(b, dtype=np.float32)
    N, Din = x.shape
    assert Din == D
    E = edge_index.shape[1]
    npc = N // NCORES
    blocks = plan_schedule(npc, win)
    nblk = len(blocks)

    src = np.asarray(edge_index[0], dtype=np.int64)
    dst = np.asarray(edge_index[1], dtype=np.int64)

    # balanced dst -> (core, slot) via degree-sorted snake deal
    deg = np.bincount(dst, minlength=N)
    order = np.argsort(-deg, kind="stable")
    win_caps, win_core, win_slotbase = [], [], []
    for c in range(NCORES):
        for bi, (ns, wins) in enumerate(blocks):
            for w, (cap, _nt) in enumerate(wins):
                win_caps.append(cap)
                win_core.append(c)
                win_slotbase.append(bi * BLK + w * win)
    win_caps = np.asarray(win_caps)
    win_core = np.asarray(win_core)
    win_slotbase = np.asarray(win_slotbase)

    core_of_dst = np.empty(N, np.int64)
    slot_of_dst = np.empty(N, np.int64)
    ptr = 0
    for r in range(win):
        act = np.flatnonzero(win_caps > r)
        if r % 2 == 1:
            act = act[::-1]
        take = order[ptr:ptr + act.size]
        ptr += act.size
        core_of_dst[take] = win_core[act]
        slot_of_dst[take] = win_slotbase[act] + r
    assert ptr == N

    dst_of_slot = np.empty((NCORES, npc), np.int64)
    dst_of_slot[core_of_dst, slot_of_dst] = np.arange(N)

    e_core = core_of_dst[dst]
    e_slot = slot_of_dst[dst]
    e_block = e_slot // BLK
    e_brel = e_slot % BLK
    e_win = e_brel // win
    e_col = e_brel % win

    maxw = max(len(wins) for _, wins in blocks)
    nt_tab = np.zeros((nblk, maxw), np.int64)
    tbase_tab = np.zeros((nblk, maxw), np.int64)
    NT1 = np.zeros(nblk, np.int64)
    for bi, (ns, wins) in enumerate(blocks):
        t = 0
        for w, (cap, nt) in enumerate(wins):
            nt_tab[bi, w] = nt
            tbase_tab[bi, w] = t
            t += nt
        NT1[bi] = t

    # rank edges within (core, block, window)
    gid = (e_core * nblk + e_block) * maxw + e_win
    order_e = np.argsort(gid, kind="stable")
    gs = gid[order_e]
    _, start_idx, counts = np.unique(gs, return_index=True, return_counts=True)
    rank = np.arange(E) - np.repeat(start_idx, counts)

    sc = e_core[order_e]
    sb = e_block[order_e]
    sw = e_win[order_e]
    scol = e_col[order_e]
    sbrel = e_brel[order_e]
    ssrc = src[order_e]

    cap_e = 128 * nt_tab[sb, sw]
    norm = rank < cap_e
    ov = ~norm

    # exact per-block straggler tile counts; always >= 1 so the block-wide
    # straggler matmul (issued first, start=True) zeroes the whole agg bank
    ovcnt = np.zeros((NCORES, nblk), np.int64)
    np.add.at(ovcnt, (sc[ov], sb[ov]), 1)
    strag_tab = np.maximum(
        np.ceil(ovcnt.max(axis=0) / 128.0).astype(np.int64), 1
    )

    gid2 = sc[ov] * nblk + sb[ov]
    if gid2.size:
        o2 = np.argsort(gid2, kind="stable")
        gid2s = gid2[o2]
        _, start2, counts2 = np.unique(gid2s, return_index=True, return_counts=True)
        rank2s = np.arange(gid2s.size) - np.repeat(start2, counts2)
        rank2 = np.empty_like(rank2s)
        rank2[o2] = rank2s
    else:
        rank2 = np.zeros(0, np.int64)

    TPB = NT1 + strag_tab
    tile_off = np.zeros(nblk + 1, np.int64)
    tile_off[1:] = np.cumsum(TPB)
    T_TOT = int(tile_off[-1])

    SB_BYTES = NT1 * win + strag_tab * BLK
    s_off = np.zeros(nblk + 1, np.int64)
    s_off[1:] = np.cumsum(SB_BYTES)
    S_TOT = int(s_off[-1])

    f8 = mybir.dt.np(F8)
    row_idx = np.full((NCORES, PART, T_TOT), N, np.int64)   # pad -> zero row
    smat_all = np.zeros((NCORES, PART, S_TOT), f8)

    tin_n = tbase_tab[sb[norm], sw[norm]] + rank[norm] // 128
    part_n = rank[norm] % 128
    tile_n = tile_off[sb[norm]] + tin_n
    row_idx[sc[norm], part_n, tile_n] = ssrc[norm]
    s64pos = s_off[sb[norm]] + tin_n * win + scol[norm]
    smat_all[sc[norm], part_n, s64pos] = 1.0

    if gid2.size:
        ks = rank2 // 128
        part_s = rank2 % 128
        tile_s = tile_off[sb[ov]] + NT1[sb[ov]] + ks
        row_idx[sc[ov], part_s, tile_s] = ssrc[ov]
        s512pos = s_off[sb[ov]] + NT1[sb[ov]] * win + ks * BLK + sbrel[ov]
        smat_all[sc[ov], part_s, s512pos] = 1.0

    # host-resolved gather: the streamed rows (fp8e3m4), zero row at index N
    x8z = np.vstack([x.astype(f8), np.zeros((1, D), f8)])
    xs_all = x8z[row_idx]                    # [NC, 128, T_TOT, D]
    xs_all = np.ascontiguousarray(xs_all.reshape(NCORES, PART, T_TOT * D))

    groups = [list(range(g, min(g + grp, nblk))) for g in range(0, nblk, grp)]

    # partition-major output layout: block b occupies out[:, out_off[b] :
    # out_off[b] + nout_b*D], slot (b*BLK + t*128 + p) at [p, t*D : (t+1)*D]
    nouts = np.array([(ns + 127) // 128 for ns, _ in blocks], np.int64)
    out_off = np.zeros(nblk + 1, np.int64)
    out_off[1:] = np.cumsum(nouts * D)

    meta = dict(
        N=N, E=E, npc=npc, nblk=nblk, blocks=blocks,
        NT1=NT1, strag_tab=strag_tab, tile_off=tile_off, s_off=s_off,
        T_TOT=T_TOT, S_TOT=S_TOT, SB_BYTES=SB_BYTES, groups=groups,
        dst_of_slot=dst_of_slot, win=win, out_f16=out_f16,
        nouts=nouts, out_off=out_off,
    )
    brep = np.broadcast_to(bias, (PART, D)).copy().astype(np.float32)
    arrs = dict(
        xs_all=xs_all, W=W.astype(np.float16), brep=brep,
        smat_all=smat_all, row_idx=row_idx,
    )
    return meta, arrs


def build_nc(meta, repeat=1, phases=("stream", "mm", "final", "out")):
    npc = meta["npc"]
    nblk = meta["nblk"]
    blocks = meta["blocks"]
    NT1 = meta["NT1"]
    strag_tab = meta["strag_tab"]
    tile_off = meta["tile_off"]
    s_off = meta["s_off"]
    T_TOT = meta["T_TOT"]
    S_TOT = meta["S_TOT"]
    SB_BYTES = meta["SB_BYTES"]
    groups = meta["groups"]
    win = meta.get("win", WIN)
    out_f16 = meta.get("out_f16", False)
    OUTDT = F16 if out_f16 else F32

    gsizes = [int(tile_off[gb[-1] + 1] - tile_off[gb[0]]) for gb in groups]
    max_gt = max(gsizes)
    max_gs = max(int(sum(SB_BYTES[b] for b in gb)) for gb in groups)

    nouts = meta["nouts"]
    out_off = meta["out_off"]
    OC = int(out_off[-1])

    nc = bacc.Bacc("TRN2", target_bir_lowering=False, debug=False)
    xs = nc.dram_tensor("xs", [PART, T_TOT * D], F8, kind="ExternalInput")
    smat = nc.dram_tensor("smat", [PART, S_TOT], F8, kind="ExternalInput")
    wmat = nc.dram_tensor("wmat", [D, D], F16, kind="ExternalInput")
    brep = nc.dram_tensor("brep", [PART, D], F32, kind="ExternalInput")
    out = nc.dram_tensor("out", [PART, OC], OUTDT, kind="ExternalOutput")

    with TileContext(nc) as tc:
        with (
            tc.tile_pool(name="const", bufs=1) as cpool,
            tc.tile_pool(name="io", bufs=3) as iopool,
            tc.tile_pool(name="xsp", bufs=2) as xsp,
            tc.tile_pool(name="outp", bufs=2) as outp,
            tc.tile_pool(name="psum", bufs=3, space="PSUM") as pp,
        ):
            w_sb = cpool.tile([PART, D], F16)
            nc.sync.dma_start(w_sb[:], wmat[:, :])
            brep_sb = cpool.tile([PART, D], F32)
            nc.sync.dma_start(brep_sb[:], brep[:, :])

            for _rep in range(repeat):
                for gi, gb in enumerate(groups):
                    gt = gsizes[gi]
                    goff = int(tile_off[gb[0]])
                    gsb = int(sum(SB_BYTES[b] for b in gb))
                    gs0 = int(s_off[gb[0]])

                    xs_t = xsp.tile([PART, max_gt * D], F8, tag="xs")
                    if "stream" in phases:
                        nc.sync.dma_start(
                            xs_t[:, :gt * D],
                            xs[:, goff * D:(goff + gt) * D],
                        )
                    s_t = iopool.tile([PART, max_gs], F8, tag="smat")
                    nc.gpsimd.dma_start(s_t[:, :gsb], smat[:, gs0:gs0 + gsb])

                    if "mm" not in phases:
                        continue
                    og0 = int(out_off[gb[0]])
                    ogw = int(out_off[gb[-1] + 1]) - og0
                    outg = outp.tile([PART, GRP * 4 * D], OUTDT, tag="out_sb")
                    for b in gb:
                        ns, wins = blocks[b]
                        nt1 = int(NT1[b])
                        bs0 = int(s_off[b]) - gs0
                        toff = int(tile_off[b]) - goff
                        ws = len(wins) * win
                        n_strag = int(strag_tab[b])

                        # one accumulation group per block: exactly one
                        # start=True (it zeroes the whole PSUM bank), carried
                        # by the first matmul issued. Stragglers go first so
                        # their block-wide [*, :ws] write doubles as the
                        # bank-zeroing pass.
                        agg = pp.tile([PART, BLK], F32, tag="agg")
                        n_mm = nt1 + n_strag
                        mm_i = 0
                        for k in range(n_strag):
                            mm_i += 1
                            ti = toff + nt1 + k
                            so = bs0 + nt1 * win + k * BLK
                            nc.tensor.matmul(
                                agg[:, :ws],
                                xs_t[:, ti * D:(ti + 1) * D],
                                s_t[:, so:so + ws],
                                start=(k == 0),
                                stop=(mm_i == n_mm),
                                skip_group_check=True,
                            )
                        for w, (cap, ntl) in enumerate(wins):
                            tb = tbase_ord(wins, w)
                            for k in range(ntl):
                                mm_i += 1
                                ti = toff + tb + k
                                nc.tensor.matmul(
                                    agg[:, w * win:w * win + win],
                                    xs_t[:, ti * D:(ti + 1) * D],
                                    s_t[:, bs0 + (tb + k) * win:
                                        bs0 + (tb + k + 1) * win],
                                    start=(mm_i == 1),
                                    stop=(mm_i == n_mm),
                                    skip_group_check=True,
                                )

                        if "final" not in phases:
                            continue
                        agg_sb = outp.tile([PART, BLK], F16, tag="agg_sb")
                        nc.vector.tensor_copy(agg_sb[:, :], agg[:, :])
                        ob = int(out_off[b]) - og0
                        nout = (ns + 127) // 128
                        for t in range(nout):
                            out2 = pp.tile([PART, D], F32, tag="out2")
                            nc.tensor.matmul(
                                out2[:, :],
                                agg_sb[:, t * PART:(t + 1) * PART],
                                w_sb[:, :],
                                start=True, stop=True,
                            )
                            nc.vector.tensor_tensor(
                                out=outg[:, ob + t * D:ob + (t + 1) * D],
                                in0=out2[:, :],
                                in1=brep_sb[:, :],
                                op=mybir.AluOpType.add,
                            )

                    if "out" not in phases or "final" not in phases:
                        continue
                    nc.scalar.dma_start(
                        out[:, og0:og0 + ogw],
                        outg[:, :ogw],
                    )
    nc.compile()
    _sanitize_waits(nc)
    return nc


def make_in_maps(meta, arrs):
    return [
        dict(
            xs=arrs["xs_all"][c],
            smat=arrs["smat_all"][c],
            wmat=arrs["W"],
            brep=arrs["brep"],
        )
        for c in range(NCORES)
    ]


def assemble_output(meta, results):
    N = meta["N"]
    npc = meta["npc"]
    blocks = meta["blocks"]
    nouts = meta["nouts"]
    out_off = meta["out_off"]
    out_full = np.empty((N, D), np.float32)
    for c in range(NCORES):
        pm = results[c]["out"]                   # [128, OC]
        rows = np.empty((npc, D), np.float32)
        for b, (ns, _w) in enumerate(blocks):
            nout = int(nouts[b])
            seg = pm[:, int(out_off[b]):int(out_off[b]) + nout * D]
            seg = seg.reshape(PART, nout, D).transpose(1, 0, 2)
            rows[b * BLK:b * BLK + ns] = seg.reshape(nout * PART, D)[:ns]
        out_full[meta["dst_of_slot"][c]] = rows
    return out_full


def kernel(x, edge_index, W, b):
    meta, arrs = pack_inputs(x, edge_index, W, b)
    nc = build_nc(meta)
    res = bass_utils.run_bass_kernel_spmd(
        nc, make_in_maps(meta, arrs), core_ids=list(range(NCORES))
    )
    return assemble_output(meta, res.results)



# revision 25
# speedup vs baseline: 1.1345x; 1.1345x over previous
"""GCN layer (segment-sum + linear) on 8 Trainium2 NeuronCores.

Reference computation:
    agg = segment_sum(x[src], dst, num_segments=N)   # (N, 128)
    out = agg @ W + b                                # (N, 128)

Strategy
--------
dst nodes are partitioned across the 8 cores (graph/data parallel), assigned
to (core, slot) by a degree-sorted snake deal so every 16-slot "window" of
every core receives a near-equal number of incoming edges. Each core
processes its 12.5K slots in blocks of 512 (one fp32 PSUM bank).

The host resolves the irregular per-edge gather at pack time: for each core
it materializes the edge-row sequence
    xs[c][p, t*128:(t+1)*128] = x_fp8e3m4[src(c, p, t)]
in exactly the (partition, tile) layout the segment-sum matmuls consume
(zero rows in pad slots), so the device streams edge rows CONTIGUOUSLY at
full HBM rate instead of issuing one small random-gather descriptor per
edge. Per 512-slot block the device runs:

  1. [128 x 512] block-wide straggler matmuls for overflow edges, issued
     FIRST — the leading one carries start=True, which zeroes the whole
     PSUM bank (every block is forced to have >= 1 straggler tile for
     exactly this reason; a second start=True inside an accumulation
     group wipes earlier columns, so there is exactly one per block),
  2. one TensorE matmul  agg[:, w*8:(w+1)*8] += Xs_tile.T @ S_tile per
     [128 x 8] window tile, where S is a host-built one-hot fp8e3m4
     matrix (S[e, j] = 1 iff edge e's dst sits at column j),
  3. the linear layer in fp16 (fp32 PSUM accum): out = agg.T @ W + b,
  4. one fp16 output DMA per 3-block group (host upcasts to fp32 after
     un-permuting).

All accumulation is fp32; the dominant rounding is fp8e3m4 (4 mantissa
bits) on x itself, ~1.3e-2 relative error against the fp32 reference —
inside the 2e-2 gate with margin, and it halves the xs HBM stream that
bounds this kernel. Queue layout keeps the xs stream unobstructed: xs
alone on the SP HWDGE queue, S on the gpsimd SWDGE queue, the output DMA
on the ACT HWDGE queue, PSUM->SBUF copies and bias adds on DVE.
"""

import math

import numpy as np

import concourse.bacc as bacc
import concourse.bass as bass
import concourse.mybir as mybir
from concourse import bass_utils
from concourse.tile import TileContext

NCORES = 8
PART = 128
BLK = 512
WIN = 8
D = 128
GRP = 2             # blocks per DMA group

F16 = mybir.dt.float16
F32 = mybir.dt.float32
F8 = mybir.dt.float8e3


def _sanitize_waits(nc):
    for bb in nc.m.functions[0].blocks:
        insts = list(bb.instructions)
        out = []
        changed = False
        for inst in insts:
            si = inst.sync_info
            waits = list(si.on_wait) if si is not None and si.on_wait else []
            if len(waits) > 1:
                changed = True
                eng = inst.engine
                for w in waits[:-1]:
                    nop = nc.engines[eng].nop(nofuse=True, hint="wsplit")
                    cb = nc.cur_bb.bb
                    cl = list(cb.instructions)
                    assert cl and cl[-1].name == nop.ins.name
                    cb.instructions = cl[:-1]
                    nop.ins.sync_info = mybir.SyncInfo(on_wait=[w], on_update=[])
                    out.append(nop.ins)
                inst.sync_info = mybir.SyncInfo(
                    on_wait=[waits[-1]], on_update=list(si.on_update or [])
                )
            out.append(inst)
        if changed:
            bb.instructions = out


def plan_schedule(npc, win=WIN):
    blocks = []
    for b0 in range(0, npc, BLK):
        ns = min(BLK, npc - b0)
        wins = []
        for w0 in range(0, ns, win):
            cap = min(win, ns - w0)
            nt = max(1, int(round(cap * 16.0 / 128.0)))
            wins.append((cap, nt))
        blocks.append((ns, wins))
    return blocks


def tbase_ord(wins, w):
    t = 0
    for i in range(w):
        t += wins[i][1]
    return t


def pack_inputs(x, edge_index, W, b, grp=GRP, win=WIN, out_f16=True):
    x = np.asarray(x)
    W = np.asarray(W, dtype=np.float32)
    bias = np.asarray(b, dtype=np.float32)
    N, Din = x.shape
    assert Din == D
    E = edge_index.shape[1]
    npc = N // NCORES
    blocks = plan_schedule(npc, win)
    nblk = len(blocks)

    src = np.asarray(edge_index[0], dtype=np.int64)
    dst = np.asarray(edge_index[1], dtype=np.int64)

    # balanced dst -> (core, slot) via degree-sorted snake deal
    deg = np.bincount(dst, minlength=N)
    order = np.argsort(-deg, kind="stable")
    win_caps, win_core, win_slotbase = [], [], []
    for c in range(NCORES):
        for bi, (ns, wins) in enumerate(blocks):
            for w, (cap, _nt) in enumerate(wins):
                win_caps.append(cap)
                win_core.append(c)
                win_slotbase.append(bi * BLK + w * win)
    win_caps = np.asarray(win_caps)
    win_core = np.asarray(win_core)
    win_slotbase = np.asarray(win_slotbase)

    core_of_dst = np.empty(N, np.int64)
    slot_of_dst = np.empty(N, np.int64)
    ptr = 0
    for r in range(win):
        act = np.flatnonzero(win_caps > r)
        if r % 2 == 1:
            act = act[::-1]
        take = order[ptr:ptr + act.size]
        ptr += act.size
        core_of_dst[take] = win_core[act]
        slot_of_dst[take] = win_slotbase[act] + r
    assert ptr == N

    dst_of_slot = np.empty((NCORES, npc), np.int64)
    dst_of_slot[core_of_dst, slot_of_dst] = np.arange(N)

    e_core = core_of_dst[dst]
    e_slot = slot_of_dst[dst]
    e_block = e_slot // BLK
    e_brel = e_slot % BLK
    e_win = e_brel // win
    e_col = e_brel % win

    maxw = max(len(wins) for _, wins in blocks)
    nt_tab = np.zeros((nblk, maxw), np.int64)
    tbase_tab = np.zeros((nblk, maxw), np.int64)
    NT1 = np.zeros(nblk, np.int64)
    for bi, (ns, wins) in enumerate(blocks):
        t = 0
        for w, (cap, nt) in enumerate(wins):
            nt_tab[bi, w] = nt
            tbase_tab[bi, w] = t
            t += nt
        NT1[bi] = t

    # rank edges within (core, block, window)
    gid = (e_core * nblk + e_block) * maxw + e_win
    order_e = np.argsort(gid, kind="stable")
    gs = gid[order_e]
    _, start_idx, counts = np.unique(gs, return_index=True, return_counts=True)
    rank = np.arange(E) - np.repeat(start_idx, counts)

    sc = e_core[order_e]
    sb = e_block[order_e]
    sw = e_win[order_e]
    scol = e_col[order_e]
    sbrel = e_brel[order_e]
    ssrc = src[order_e]

    cap_e = 128 * nt_tab[sb, sw]
    norm = rank < cap_e
    ov = ~norm

    # exact per-block straggler tile counts; always >= 1 so the block-wide
    # straggler matmul (issued first, start=True) zeroes the whole agg bank
    ovcnt = np.zeros((NCORES, nblk), np.int64)
    np.add.at(ovcnt, (sc[ov], sb[ov]), 1)
    strag_tab = np.maximum(
        np.ceil(ovcnt.max(axis=0) / 128.0).astype(np.int64), 1
    )

    gid2 = sc[ov] * nblk + sb[ov]
    if gid2.size:
        o2 = np.argsort(gid2, kind="stable")
        gid2s = gid2[o2]
        _, start2, counts2 = np.unique(gid2s, return_index=True, return_counts=True)
        rank2s = np.arange(gid2s.size) - np.repeat(start2, counts2)
        rank2 = np.empty_like(rank2s)
        rank2[o2] = rank2s
    else:
        rank2 = np.zeros(0, np.int64)

    TPB = NT1 + strag_tab
    tile_off = np.zeros(nblk + 1, np.int64)
    tile_off[1:] = np.cumsum(TPB)
    T_TOT = int(tile_off[-1])

    SB_BYTES = NT1 * win + strag_tab * BLK
    s_off = np.zeros(nblk + 1, np.int64)
    s_off[1:] = np.cumsum(SB_BYTES)
    S_TOT = int(s_off[-1])

    f8 = mybir.dt.np(F8)
    row_idx = np.full((NCORES, PART, T_TOT), N, np.int64)   # pad -> zero row
    smat_all = np.zeros((NCORES, PART, S_TOT), f8)

    tin_n = tbase_tab[sb[norm], sw[norm]] + rank[norm] // 128
    part_n = rank[norm] % 128
    tile_n = tile_off[sb[norm]] + tin_n
    row_idx[sc[norm], part_n, tile_n] = ssrc[norm]
    s64pos = s_off[sb[norm]] + tin_n * win + scol[norm]
    smat_all[sc[norm], part_n, s64pos] = 1.0

    if gid2.size:
        ks = rank2 // 128
        part_s = rank2 % 128
        tile_s = tile_off[sb[ov]] + NT1[sb[ov]] + ks
        row_idx[sc[ov], part_s, tile_s] = ssrc[ov]
        s512pos = s_off[sb[ov]] + NT1[sb[ov]] * win + ks * BLK + sbrel[ov]
        smat_all[sc[ov], part_s, s512pos] = 1.0

    # host-resolved gather: the streamed rows (fp8e3m4), zero row at index N
    x8z = np.vstack([x.astype(f8), np.zeros((1, D), f8)])
    xs_all = x8z[row_idx]                    # [NC, 128, T_TOT, D]
    xs_all = np.ascontiguousarray(xs_all.reshape(NCORES, PART, T_TOT * D))

    groups = [list(range(g, min(g + grp, nblk))) for g in range(0, nblk, grp)]

    # partition-major output layout: block b occupies out[:, out_off[b] :
    # out_off[b] + nout_b*D], slot (b*BLK + t*128 + p) at [p, t*D : (t+1)*D]
    nouts = np.array([(ns + 127) // 128 for ns, _ in blocks], np.int64)
    out_off = np.zeros(nblk + 1, np.int64)
    out_off[1:] = np.cumsum(nouts * D)

    meta = dict(
        N=N, E=E, npc=npc, nblk=nblk, blocks=blocks,
        NT1=NT1, strag_tab=strag_tab, tile_off=tile_off, s_off=s_off,
        T_TOT=T_TOT, S_TOT=S_TOT, SB_BYTES=SB_BYTES, groups=groups,
        dst_of_slot=dst_of_slot, win=win, out_f16=out_f16,
        nouts=nouts, out_off=out_off,
    )
    brep = np.broadcast_to(bias, (PART, D)).copy().astype(np.float32)
    arrs = dict(
        xs_all=xs_all, W=W.astype(np.float16), brep=brep,
        smat_all=smat_all, row_idx=row_idx,
    )
    return meta, arrs


def build_nc(meta, repeat=1, phases=("stream", "mm", "final", "out")):
    npc = meta["npc"]
    nblk = meta["nblk"]
    blocks = meta["blocks"]
    NT1 = meta["NT1"]
    strag_tab = meta["strag_tab"]
    tile_off = meta["tile_off"]
    s_off = meta["s_off"]
    T_TOT = meta["T_TOT"]
    S_TOT = meta["S_TOT"]
    SB_BYTES = meta["SB_BYTES"]
    groups = meta["groups"]
    win = meta.get("win", WIN)
    out_f16 = meta.get("out_f16", False)
    OUTDT = F16 if out_f16 else F32

    gsizes = [int(tile_off[gb[-1] + 1] - tile_off[gb[0]]) for gb in groups]
    max_gt = max(gsizes)
    max_gs = max(int(sum(SB_BYTES[b] for b in gb)) for gb in groups)

    nouts = meta["nouts"]
    out_off = meta["out_off"]
    OC = int(out_off[-1])

    nc = bacc.Bacc("TRN2", target_bir_lowering=False, debug=False)
    xs = nc.dram_tensor("xs", [PART, T_TOT * D], F8, kind="ExternalInput")
    smat = nc.dram_tensor("smat", [PART, S_TOT], F8, kind="ExternalInput")
    wmat = nc.dram_tensor("wmat", [D, D], F16, kind="ExternalInput")
    brep = nc.dram_tensor("brep", [PART, D], F32, kind="ExternalInput")
    out = nc.dram_tensor("out", [PART, OC], OUTDT, kind="ExternalOutput")

    with TileContext(nc) as tc:
        with (
            tc.tile_pool(name="const", bufs=1) as cpool,
            tc.tile_pool(name="io", bufs=3) as iopool,
            tc.tile_pool(name="xsp", bufs=3) as xsp,
            tc.tile_pool(name="outp", bufs=2) as outp,
            tc.tile_pool(name="psum", bufs=3, space="PSUM") as pp,
        ):
            w_sb = cpool.tile([PART, D], F16)
            nc.sync.dma_start(w_sb[:], wmat[:, :])
            brep_sb = cpool.tile([PART, D], F32)
            nc.sync.dma_start(brep_sb[:], brep[:, :])

            for _rep in range(repeat):
                for gi, gb in enumerate(groups):
                    gt = gsizes[gi]
                    goff = int(tile_off[gb[0]])
                    gsb = int(sum(SB_BYTES[b] for b in gb))
                    gs0 = int(s_off[gb[0]])

                    xs_t = xsp.tile([PART, max_gt * D], F8, tag="xs")
                    if "stream" in phases:
                        nc.sync.dma_start(
                            xs_t[:, :gt * D],
                            xs[:, goff * D:(goff + gt) * D],
                        )
                    s_t = iopool.tile([PART, max_gs], F8, tag="smat")
                    nc.gpsimd.dma_start(s_t[:, :gsb], smat[:, gs0:gs0 + gsb])

                    if "mm" not in phases:
                        continue
                    og0 = int(out_off[gb[0]])
                    ogw = int(out_off[gb[-1] + 1]) - og0
                    outg = outp.tile([PART, GRP * 4 * D], OUTDT, tag="out_sb")
                    for b in gb:
                        ns, wins = blocks[b]
                        nt1 = int(NT1[b])
                        bs0 = int(s_off[b]) - gs0
                        toff = int(tile_off[b]) - goff
                        ws = len(wins) * win
                        n_strag = int(strag_tab[b])

                        # one accumulation group per block: exactly one
                        # start=True (it zeroes the whole PSUM bank), carried
                        # by the first matmul issued. Stragglers go first so
                        # their block-wide [*, :ws] write doubles as the
                        # bank-zeroing pass.
                        agg = pp.tile([PART, BLK], F32, tag="agg")
                        n_mm = nt1 + n_strag
                        mm_i = 0
                        for k in range(n_strag):
                            mm_i += 1
                            ti = toff + nt1 + k
                            so = bs0 + nt1 * win + k * BLK
                            nc.tensor.matmul(
                                agg[:, :ws],
                                xs_t[:, ti * D:(ti + 1) * D],
                                s_t[:, so:so + ws],
                                start=(k == 0),
                                stop=(mm_i == n_mm),
                                skip_group_check=True,
                            )
                        for w, (cap, ntl) in enumerate(wins):
                            tb = tbase_ord(wins, w)
                            for k in range(ntl):
                                mm_i += 1
                                ti = toff + tb + k
                                nc.tensor.matmul(
                                    agg[:, w * win:w * win + win],
                                    xs_t[:, ti * D:(ti + 1) * D],
                                    s_t[:, bs0 + (tb + k) * win:
                                        bs0 + (tb + k + 1) * win],
                                    start=(mm_i == 1),
                                    stop=(mm_i == n_mm),
                                    skip_group_check=True,
                                )

                        if "final" not in phases:
                            continue
                        agg_sb = outp.tile([PART, BLK], F16, tag="agg_sb")
                        nc.vector.tensor_copy(agg_sb[:, :], agg[:, :])
                        ob = int(out_off[b]) - og0
                        nout = (ns + 127) // 128
                        for t in range(nout):
                            out2 = pp.tile([PART, D], F32, tag="out2")
                            nc.tensor.matmul(
                                out2[:, :],
                                agg_sb[:, t * PART:(t + 1) * PART],
                                w_sb[:, :],
                                start=True, stop=True,
                            )
                            nc.vector.tensor_tensor(
                                out=outg[:, ob + t * D:ob + (t + 1) * D],
                                in0=out2[:, :],
                                in1=brep_sb[:, :],
                                op=mybir.AluOpType.add,
                            )

                    if "out" not in phases or "final" not in phases:
                        continue
                    nc.scalar.dma_start(
                        out[:, og0:og0 + ogw],
                        outg[:, :ogw],
                    )
    nc.compile()
    _sanitize_waits(nc)
    return nc


def make_in_maps(meta, arrs):
    return [
        dict(
            xs=arrs["xs_all"][c],
            smat=arrs["smat_all"][c],
            wmat=arrs["W"],
            brep=arrs["brep"],
        )
        for c in range(NCORES)
    ]


def assemble_output(meta, results):
    N = meta["N"]
    npc = meta["npc"]
    blocks = meta["blocks"]
    nouts = meta["nouts"]
    out_off = meta["out_off"]
    out_full = np.empty((N, D), np.float32)
    for c in range(NCORES):
        pm = results[c]["out"]                   # [128, OC]
        rows = np.empty((npc, D), np.float32)
        for b, (ns, _w) in enumerate(blocks):
            nout = int(nouts[b])
            seg = pm[:, int(out_off[b]):int(out_off[b]) + nout * D]
            seg = seg.reshape(PART, nout, D).transpose(1, 0, 2)
            rows[b * BLK:b * BLK + ns] = seg.reshape(nout * PART, D)[:ns]
        out_full[meta["dst_of_slot"][c]] = rows
    return out_full


def kernel(x, edge_index, W, b):
    meta, arrs = pack_inputs(x, edge_index, W, b)
    nc = build_nc(meta)
    res = bass_utils.run_bass_kernel_spmd(
        nc, make_in_maps(meta, arrs), core_ids=list(range(NCORES))
    )
    return assemble_output(meta, res.results)

